# revision 2
# baseline (speedup 1.0000x reference)
"""Trainium2 Bass kernel for nn_MessagePassing_42588895707817.

out = (h @ W.T + b) @ norm_graph,  norm_graph = graph / clip(rowsum(graph), EPS)

Math folding: out = h @ C + d  with  C = W.T @ norm_graph  (128x128),
d = b @ norm_graph (zeros here; handled exactly on host if nonzero).

Device does ONLY the big streaming matmul, in bf16 (rel-err budget 2e-2
dwarfs bf16's ~0.5%): per core 32768 tokens x 128 feat.  The host
pre-transposes h to [f, tok] tiles so the PE needs no on-chip transpose:
matmul(out_T[g, tok], lhsT=C[f, g], rhs=hT[f, tok]).  HBM traffic per
core is 8.4 MB in + 8.4 MB out (vs 33.5 MB for the fp32+transpose
version) -> ~47 us roofline at 358 GB/s.

Sharding: data-parallel on batch B=32 across 8 cores (4 batches/core).
"""

import sys

if "/opt/trn_rl_repo" not in sys.path:
    sys.path.insert(0, "/opt/trn_rl_repo")

from contextlib import ExitStack

import ml_dtypes
import numpy as np

B, T, FDIM, HID = 32, 8192, 128, 128
EPS = 1e-10
NCORES = 8
B_LOC = B // NCORES
NTOK = B_LOC * T  # 32768 tokens per core

P = 128  # partitions
MMN = 512  # matmul free dim (one PSUM bank of fp32)


def build_program(ntok=NTOK, ch=4096):
    import concourse.bacc as bacc
    import concourse.tile as tile
    from concourse import mybir

    f32 = mybir.dt.float32
    bf16 = mybir.dt.bfloat16
    nchunk = ntok // ch
    nmm = ch // MMN
    assert ntok % ch == 0 and ch % MMN == 0

    nc = bacc.Bacc("TRN2", debug=False, target_bir_lowering=False)

    ht_d = nc.dram_tensor("ht", [nchunk, P, ch], bf16, kind="ExternalInput")
    c_d = nc.dram_tensor("C", [P, P], bf16, kind="ExternalInput")
    out_d = nc.dram_tensor("out", [nchunk, P, ch], bf16, kind="ExternalOutput")

    with tile.TileContext(nc) as tc, ExitStack() as ctx:
        singles = ctx.enter_context(tc.tile_pool(name="singles", bufs=1))
        ld = ctx.enter_context(tc.tile_pool(name="ld", bufs=3))
        st = ctx.enter_context(tc.tile_pool(name="st", bufs=3))
        ps = ctx.enter_context(tc.tile_pool(name="ps", bufs=8, space="PSUM"))

        c_raw = singles.tile([P, P], bf16)
        nc.sync.dma_start(out=c_raw, in_=c_d[:])
        # Stage C through DVE so matmuls only ever wait on one sem each.
        c_s = singles.tile([P, P], bf16)
        nc.vector.tensor_copy(c_s, c_raw)

        k = 0
        for c in range(nchunk):
            in_t = ld.tile([P, ch], bf16)
            nc.sync.dma_start(out=in_t, in_=ht_d[c])
            out_t = st.tile([P, ch], bf16)
            for j in range(nmm):
                o_ps = ps.tile([P, MMN], f32)
                nc.tensor.matmul(o_ps, lhsT=c_s, rhs=in_t[:, j * MMN:(j + 1) * MMN],
                                 start=True, stop=True)
                dst = out_t[:, j * MMN:(j + 1) * MMN]
                if k % 2 == 0:
                    nc.vector.tensor_copy(dst, o_ps)
                else:
                    nc.scalar.copy(dst, o_ps)
                k += 1
            nc.scalar.dma_start(out=out_d[c], in_=out_t)

    nc.compile()
    return nc


def _fold_constants(graph, W, b):
    """C = W.T @ norm_graph (bf16), d = b @ norm_graph (fp32, exact path)."""
    g = np.asarray(graph, dtype=np.float64)
    deg = np.clip(g.sum(axis=1, keepdims=True), EPS, None)
    norm = np.where(deg > EPS, g / deg, 0.0)
    C = (np.asarray(W, dtype=np.float64).T @ norm).astype(ml_dtypes.bfloat16)
    d = (np.asarray(b, dtype=np.float64) @ norm).astype(np.float32)
    return C, d


def make_in_maps(h, graph, W, b, ch=4096):
    nchunk = NTOK // ch
    C, _ = _fold_constants(graph, W, b)
    hb = np.asarray(h, dtype=np.float32).reshape(NCORES, NTOK, FDIM)
    hb = hb.astype(ml_dtypes.bfloat16)
    return [
        {
            "ht": np.ascontiguousarray(
                hb[i].reshape(nchunk, ch, FDIM).transpose(0, 2, 1)
            ),
            "C": C,
        }
        for i in range(NCORES)
    ]


def unpack_outputs(res, b_d, ch=4096):
    nchunk = NTOK // ch
    outs = []
    for i in range(NCORES):
        r = res.results[i]["out"].reshape(nchunk, HID, ch)
        o = r.transpose(0, 2, 1).reshape(B_LOC, T, HID).astype(np.float32)
        outs.append(o)
    out = np.concatenate(outs, axis=0)
    if b_d is not None:
        out = out + b_d[None, None, :]
    return out


_LDW_PATCHED = False


def _enable_ldw_opt(bass_utils):
    """Compile walrus with --enable-ldw-opt=true: lets the PE hide LDWEIGHTS
    behind in-flight matmuls."""
    global _LDW_PATCHED
    if _LDW_PATCHED:
        return
    _LDW_PATCHED = True
    orig = bass_utils.run_command

    def patched(argv, **kw):
        argv = [a.replace("--enable-ldw-opt=false", "--enable-ldw-opt=true")
                if isinstance(a, str) else a for a in argv]
        return orig(argv, **kw)

    bass_utils.run_command = patched


def kernel(h, graph, W, b):
    from concourse import bass_utils

    _enable_ldw_opt(bass_utils)
    ch = 4096
    nc = build_program(ch=ch)
    in_maps = make_in_maps(h, graph, W, b, ch=ch)
    res = bass_utils.run_bass_kernel_spmd(nc, in_maps, list(range(NCORES)))
    b_np = np.asarray(b, dtype=np.float64)
    d = _fold_constants(graph, W, b)[1] if np.any(b_np) else None
    return unpack_outputs(res, d, ch=ch)
